# revision 18
# baseline (speedup 1.0000x reference)
"""Trainium2 Bass kernel for DifferentiableBoxParser.

Per (b, k): softmax over the 256x256 score map (T=0.1) -> expected coords
(y, x); soft-ceil + smooth-clamp; int cast; gather offsets at the resulting
index; pts = (coords + offset) * 4.

Device does the heavy part (streaming the 128 MiB score_map and computing,
per map, the softmax partial sums Z, Sy-parts, Sx). Host finishes the tiny
per-pair scalar math and the 2-element-per-pair offset gather (reading the
256 MiB offset_map on device would be pure waste: only 1024 of its elements
are needed).

Sharding: data-parallel over batch, 8 batches per core (64 maps per core).

Device layout per core: score reshaped to [1024, 4096]; group g in [0,8)
covers 8 maps; SBUF tile [128, 4096] with partition p = 16*j + s (j = map in
group, s = h-high), free f = h_low*256 + w with h = 16*s + h_low. Per 512-col
chunk q (h_low = 2q + b, b = (f%512)//256), a matmul with block-diagonal
weights accumulates into PSUM [16, 512]:
  row 2j   : colsum_j[f']  = sum_s E
  row 2j+1 : sum_s (16s + 2q) E
Finalize per group on DVE: Z = sum(row 2j); B = sum(row 2j, f' in [256,512));
S16 = sum(row 2j+1); Xw = sum(f'%256 * row 2j).
Then y = (S16 + B)/Z, x = Xw/Z on host. exp computed as exp(10*x - 40)
(softmax is shift-invariant; keeps f32 range safe).

Matmuls run in float32r (fast PE mode, ~1.3e-4 relative error). The neuron
jax backend rounds float->int casts half-to-even, so pairs whose clamped
coords land within REFINE_DELTA of a half-integer boundary are recomputed
exactly on host in float64 so the non-differentiable cast can't flip.
"""
import sys
import numpy as np

for _p in ("/opt/trn_rl_repo", "/opt/pypackages"):
    if _p not in sys.path:
        sys.path.append(_p)

import concourse.bacc as bacc
import concourse.tile as tile
from concourse import mybir
from concourse.bass_utils import run_bass_kernel_spmd

N_CORES = 8
BS, K, HO, WO = 64, 8, 256, 256
STRIDE = 4
TEMPERATURE = 0.1
SHARPNESS = 10.0
SMOOTHNESS = 0.1
EXP_SHIFT = -40.0

NPG = 8            # maps per group
NGROUP = 8         # groups per core (8 maps/group * 8 groups = 64 maps/core)
P = 128
FD = 4096
NCHUNK = 8
MM_DT = mybir.dt.float32r
REFINE_DELTA = 0.05

_CACHE = {}


def _build_nc():
    nc = bacc.Bacc(None, target_bir_lowering=False, debug=False)
    score = nc.dram_tensor("score", [NGROUP * P, FD], mybir.dt.float32,
                           kind="ExternalInput")
    wmat = nc.dram_tensor("wmat", [P, NCHUNK + 16, 16], MM_DT, kind="ExternalInput")
    wvin = nc.dram_tensor("wvin", [16, 2, 256], mybir.dt.float32, kind="ExternalInput")
    # slots 0..6: groups 0..6; slot 7/8: last group's first/second PSUM
    # window (host adds them -- Z, S16, Xw are all plain sums)
    stats = nc.dram_tensor("stats", [16, NGROUP + 1, 3], mybir.dt.float32,
                           kind="ExternalOutput")

    with tile.TileContext(nc) as tc:
        with (
            tc.tile_pool(name="singles", bufs=1) as singles,
            tc.tile_pool(name="xin", bufs=4) as xin,
            tc.tile_pool(name="xlast", bufs=1) as xlast,
            tc.tile_pool(name="expo", bufs=3) as expo,
            tc.tile_pool(name="psum", bufs=4, space="PSUM") as psum_pool,
        ):
            # startup dead zone (engine barrier + instruction fetch lasts
            # ~7us before the first score byte lands): memsets and a warmup
            # exp that pulls the ACT_TABLE_LOAD off the first real exp's
            # critical path
            bias_t = singles.tile([P, 1], mybir.dt.float32)
            nc.vector.memset(bias_t[:], EXP_SHIFT)
            warm = singles.tile([P, 1], mybir.dt.float32)
            nc.scalar.activation(out=warm[:], in_=bias_t[:],
                                 func=mybir.ActivationFunctionType.Exp,
                                 bias=bias_t[:], scale=1.0)
            # all per-group stats accumulate here; one DMA ships them at the
            # end (one sem slot instead of eight)
            oball = singles.tile([16, NGROUP + 1, 3], mybir.dt.float32)
            nc.vector.memset(oball[:], 0.0)
            tmp = singles.tile([16, 512], mybir.dt.float32)
            junk = singles.tile([16, 256], mybir.dt.float32)

            # weights go through the idle gpsimd SWDGE so the sync ring's
            # very first descriptor is score data (PE doesn't need them
            # until ~16us in)
            wt = singles.tile([P, NCHUNK + 16, 16], MM_DT)
            nc.gpsimd.dma_start(out=wt[:], in_=wmat[:])
            wvec = singles.tile([16, 2, 256], mybir.dt.float32)
            nc.gpsimd.dma_start(out=wvec[:], in_=wvin[:])

            xts = []
            for g in range(2):
                xt = xin.tile([P, FD], mybir.dt.float32)
                nc.sync.dma_start(out=xt[:], in_=score[g * P:(g + 1) * P, :])
                xts.append(xt)

            wv256 = wvec.rearrange("p a b -> p (a b)")  # [16, 512] = iota%256

            def finalize_zx(ps_sl, slot, cw, use_act):
                """Z/S16 + Xw of one PSUM window into oball[:, slot, :].

                use_act: Z via ACT Copy+accum (parallel with the DVE chain;
                right for the drain, where ACT is otherwise done). Mid-stream
                ACT must keep chasing exps, so window A sums Z on DVE."""
                # DVE op first in program order: the scheduler sequences
                # same-PSUM readers by emission order, and the DVE leg is
                # the longer one — emitting it first lets ACT's Copy+accum
                # run concurrently instead of gating it
                nc.vector.tensor_mul(tmp[:, 0:cw], ps_sl, wv256[:, 0:cw])
                nc.vector.reduce_sum(oball[:, slot, 2:3], tmp[:, 0:cw],
                                     axis=mybir.AxisListType.X)
                if use_act:
                    nc.scalar.activation(out=junk[:, 0:cw], in_=ps_sl,
                                         func=mybir.ActivationFunctionType.Copy,
                                         accum_out=oball[:, slot, 0:1])
                else:
                    nc.vector.reduce_sum(oball[:, slot, 0:1], ps_sl,
                                         axis=mybir.AxisListType.X)

            for g in range(NGROUP - 1):
                if g < 2:
                    xt = xts[g]
                else:
                    xt = xin.tile([P, FD], mybir.dt.float32)
                    nc.sync.dma_start(out=xt[:], in_=score[g * P:(g + 1) * P, :])
                et = expo.tile([P, FD], MM_DT)
                ps = psum_pool.tile([16, 512], mybir.dt.float32)
                cw = FD // NCHUNK
                for a in range(2):
                    astep = FD // 2
                    nc.scalar.activation(out=et[:, a * astep:(a + 1) * astep],
                                         in_=xt[:, a * astep:(a + 1) * astep],
                                         func=mybir.ActivationFunctionType.Exp,
                                         bias=bias_t[:], scale=1.0 / TEMPERATURE)
                    for q in range(a * 4, a * 4 + 4):
                        nc.tensor.matmul(
                            ps[:, 0:cw], wt[:, q, :], et[:, q * cw:(q + 1) * cw],
                            start=(q == 0), stop=(q == NCHUNK - 1),
                        )
                nc.vector.reduce_sum(oball[:, g, 0:1], ps[:, 0:cw],
                                     axis=mybir.AxisListType.X)
                nc.vector.reduce_sum(oball[:, g, 1:2], ps[:, 256:512],
                                     axis=mybir.AxisListType.X)
                nc.vector.tensor_mul(tmp[:, 0:cw], ps[:, 0:cw], wv256[:, 0:cw])
                nc.vector.reduce_sum(oball[:, g, 2:3], tmp[:, 0:cw],
                                     axis=mybir.AxisListType.X)

            # Last group: lands in 512-col pieces (own pool: fresh sem slots,
            # so no anti-alias wait on g5/g6 completion delays the descriptor
            # issue and starves the engines), final piece split 256+256 so the
            # post-stream drain holds only one 256-col exp. 256-wide matmul
            # chunks whose weights absorb h_low entirely (no B term), split
            # into two PSUM accumulation windows: window A (chunks 0-7)
            # finalizes mid-stream into slot 7, window B (chunks 8-15)
            # finalizes in the drain into slot 8; the host adds the slots.
            g = NGROUP - 1
            xt = xlast.tile([P, FD], mybir.dt.float32)
            bounds = [0, 512, 1024, 1536, 2048, 2560, 3072, 3584, 3840, 4096]
            for dd in range(len(bounds) - 1):
                nc.sync.dma_start(out=xt[:, bounds[dd]:bounds[dd + 1]],
                                  in_=score[g * P:(g + 1) * P,
                                            bounds[dd]:bounds[dd + 1]])
            et = expo.tile([P, FD], MM_DT)
            ps7 = psum_pool.tile([16, 512], mybir.dt.float32)
            psA = ps7[:, 0:256]
            psB = ps7[:, 256:512]
            for a in range(len(bounds) - 1):
                lo, hi = bounds[a], bounds[a + 1]
                nc.scalar.activation(out=et[:, lo:hi], in_=xt[:, lo:hi],
                                     func=mybir.ActivationFunctionType.Exp,
                                     bias=bias_t[:], scale=1.0 / TEMPERATURE)
                for q in range(lo // 256, hi // 256):
                    nc.tensor.matmul(
                        psA if q < 8 else psB, wt[:, NCHUNK + q, :],
                        et[:, q * 256:(q + 1) * 256],
                        start=(q % 8 == 0), stop=(q % 8 == 7),
                    )
                if hi == 2048:
                    finalize_zx(psA, NGROUP - 1, 256, use_act=False)
            finalize_zx(psB, NGROUP, 256, use_act=True)

            nc.sync.dma_start(out=stats[:], in_=oball[:])

    nc.compile()
    return nc


def _weights():
    W = np.zeros((P, NCHUNK + 16, 16), dtype=np.float32)
    s = np.arange(16)
    for j in range(NPG):
        W[16 * j + s, :, 2 * j] = 1.0
        for q in range(NCHUNK):          # 512-wide chunks: h_low = 2q + b
            W[16 * j + s, q, 2 * j + 1] = (16 * s + 2 * q).astype(np.float32)
        for q in range(16):              # 256-wide chunks: h_low = q
            W[16 * j + s, NCHUNK + q, 2 * j + 1] = (16 * s + q).astype(np.float32)
    return W


def _get_compiled():
    if "nc" not in _CACHE:
        _CACHE["nc"] = _build_nc()
        _CACHE["W"] = _weights()
        wv = np.tile(np.arange(256, dtype=np.float32)[None, None, :], (16, 2, 1))
        _CACHE["WV"] = wv
    return _CACHE["nc"], _CACHE["W"]


def _device_coords(score_map, trace=False):
    """Run the Bass kernel on 8 cores; return y, x arrays of shape [BS, K]
    (plus the BassKernelResults of the run)."""
    nc, W = _get_compiled()
    flat = np.ascontiguousarray(score_map.reshape(BS * K, 16, FD))
    bpc = BS // N_CORES                      # batches per core
    mpc = bpc * K                            # maps per core
    in_maps = []
    for c in range(N_CORES):
        shard = flat[c * mpc:(c + 1) * mpc].reshape(NGROUP * P, FD)
        in_maps.append({"score": shard, "wmat": W, "wvin": _CACHE["WV"]})
    res = run_bass_kernel_spmd(nc, in_maps, list(range(N_CORES)), trace=trace)

    ys = np.empty((BS, K), dtype=np.float64)
    xs = np.empty((BS, K), dtype=np.float64)
    for c in range(N_CORES):
        st = res.results[c]["stats"].astype(np.float64)   # [16, 9, 3]
        rows = np.arange(NPG)
        # slot 8 is the last group's second PSUM window; fold it into slot 7
        st = st[:, :NGROUP, :] + np.pad(
            st[:, NGROUP:, :], ((0, 0), (NGROUP - 1, 0), (0, 0)))
        Z = st[2 * rows, :, 0]               # [j, g]
        S16 = st[2 * rows + 1, :, 0]
        B = st[2 * rows, :, 1].copy()
        B[:, NGROUP - 1] = 0.0           # last group: h_low fully in S16
        Xw = st[2 * rows, :, 2]
        y = (S16 + B) / Z                    # [j, g]
        x = Xw / Z
        # map (g, j) -> core-local pair index 8g + j -> (b_local, k)
        y = y.T.reshape(mpc)                 # [g, j] -> pair-major
        x = x.T.reshape(mpc)
        ys[c * bpc:(c + 1) * bpc] = y.reshape(bpc, K)
        xs[c * bpc:(c + 1) * bpc] = x.reshape(bpc, K)
    return ys, xs, res


def _exact_coords(sm64):
    """Float64 softmax expected coords for one [HO, WO] score map."""
    z = sm64 / TEMPERATURE
    z = z - z.max()
    e = np.exp(z)
    Z = e.sum()
    y = (e.sum(axis=1) * np.arange(HO)).sum() / Z
    x = (e.sum(axis=0) * np.arange(WO)).sum() / Z
    return y, x


def _soft_ceil(x):
    return x + (1.0 - 1.0 / (1.0 + np.exp(-SHARPNESS * (x - np.floor(x)))))


def _smooth_clamp(x, min_val, max_val):
    x = np.where(x < min_val,
                 min_val + SMOOTHNESS * np.tanh((x - min_val) / SMOOTHNESS), x)
    x = np.where(x > max_val,
                 max_val - SMOOTHNESS * np.tanh((max_val - x) / SMOOTHNESS), x)
    return x


def kernel(score_map, offset_map, _trace=False, _res_out=None):
    score_map = np.asarray(score_map)
    offset_map = np.asarray(offset_map)

    ys, xs, res = _device_coords(score_map, trace=_trace)
    if _res_out is not None:
        _res_out.append(res)

    coords = np.stack([ys, xs], axis=-1)          # [BS, K, 2] float64

    cc = _soft_ceil(coords)
    y_cl = _smooth_clamp(cc[..., 0], 0.0, float(HO - 1))
    x_cl = _smooth_clamp(cc[..., 1], 0.0, float(WO - 1))

    # The harness executes the reference on the same neuron jax backend,
    # where .astype(int32) rounds half-to-even (np.rint) rather than
    # truncating. Refine pairs whose clamped coords sit near a rounding
    # boundary (half-integers): the cast there is sensitive to the
    # device's float32r noise.
    fy = np.abs(y_cl - np.floor(y_cl) - 0.5)
    fx = np.abs(x_cl - np.floor(x_cl) - 0.5)
    sus = (fy < REFINE_DELTA) | (fx < REFINE_DELTA)
    for b, k in zip(*np.nonzero(sus)):
        yy, xx = _exact_coords(score_map[b, k].astype(np.float64))
        coords[b, k, 0] = yy
        coords[b, k, 1] = xx
        cc = _soft_ceil(coords[b, k])
        y_cl[b, k] = _smooth_clamp(cc[0], 0.0, float(HO - 1))
        x_cl[b, k] = _smooth_clamp(cc[1], 0.0, float(WO - 1))

    y_idx = np.rint(y_cl).astype(np.int32)
    x_idx = np.rint(x_cl).astype(np.int32)
    b_idx = np.arange(BS)[:, None]
    k_idx = np.arange(K)[None, :]
    off_y = offset_map[b_idx, 2 * k_idx, y_idx, x_idx]
    off_x = offset_map[b_idx, 2 * k_idx + 1, y_idx, x_idx]
    offset = np.stack([off_y, off_x], axis=-1)

    pts = (coords.astype(np.float32) + offset) * STRIDE
    return pts.astype(np.float32)



# revision 19
# speedup vs baseline: 1.0252x; 1.0252x over previous
"""Trainium2 Bass kernel for DifferentiableBoxParser.

Per (b, k): softmax over the 256x256 score map (T=0.1) -> expected coords
(y, x); soft-ceil + smooth-clamp; int cast; gather offsets at the resulting
index; pts = (coords + offset) * 4.

Device does the heavy part (streaming the 128 MiB score_map and computing,
per map, the softmax partial sums Z, Sy-parts, Sx). Host finishes the tiny
per-pair scalar math and the 2-element-per-pair offset gather (reading the
256 MiB offset_map on device would be pure waste: only 1024 of its elements
are needed).

Sharding: data-parallel over batch, 8 batches per core (64 maps per core).

Device layout per core: score reshaped to [1024, 4096]; group g in [0,8)
covers 8 maps; SBUF tile [128, 4096] with partition p = 16*j + s (j = map in
group, s = h-high), free f = h_low*256 + w with h = 16*s + h_low. Per 512-col
chunk q (h_low = 2q + b, b = (f%512)//256), a matmul with block-diagonal
weights accumulates into PSUM [16, 512]:
  row 2j   : colsum_j[f']  = sum_s E
  row 2j+1 : sum_s (16s + 2q) E
Finalize per group on DVE: Z = sum(row 2j); B = sum(row 2j, f' in [256,512));
S16 = sum(row 2j+1); Xw = sum(f'%256 * row 2j).
Then y = (S16 + B)/Z, x = Xw/Z on host. exp computed as exp(10*x - 40)
(softmax is shift-invariant; keeps f32 range safe).

Matmuls run in float32r (fast PE mode, ~1.3e-4 relative error). The neuron
jax backend rounds float->int casts half-to-even, so pairs whose clamped
coords land within REFINE_DELTA of a half-integer boundary are recomputed
exactly on host in float64 so the non-differentiable cast can't flip.
"""
import sys
import numpy as np

for _p in ("/opt/trn_rl_repo", "/opt/pypackages"):
    if _p not in sys.path:
        sys.path.append(_p)

import concourse.bacc as bacc
import concourse.tile as tile
from concourse import mybir
from concourse.bass_utils import run_bass_kernel_spmd

N_CORES = 8
BS, K, HO, WO = 64, 8, 256, 256
STRIDE = 4
TEMPERATURE = 0.1
SHARPNESS = 10.0
SMOOTHNESS = 0.1
EXP_SHIFT = -40.0

NPG = 8            # maps per group
NGROUP = 8         # groups per core (8 maps/group * 8 groups = 64 maps/core)
P = 128
FD = 4096
NCHUNK = 8
MM_DT = mybir.dt.float32r
REFINE_DELTA = 0.05

_CACHE = {}


def _build_nc():
    nc = bacc.Bacc(None, target_bir_lowering=False, debug=False)
    score = nc.dram_tensor("score", [NGROUP * P, FD], mybir.dt.float32,
                           kind="ExternalInput")
    wmat = nc.dram_tensor("wmat", [P, NCHUNK + 16, 16], MM_DT, kind="ExternalInput")
    wvin = nc.dram_tensor("wvin", [16, 2, 256], mybir.dt.float32, kind="ExternalInput")
    # slots 0..6: groups 0..6; slot 7/8: last group's first/second PSUM
    # window (host adds them -- Z, S16, Xw are all plain sums)
    stats = nc.dram_tensor("stats", [16, NGROUP + 1, 3], mybir.dt.float32,
                           kind="ExternalOutput")

    with tile.TileContext(nc) as tc:
        with (
            tc.tile_pool(name="singles", bufs=1) as singles,
            tc.tile_pool(name="xin", bufs=4) as xin,
            tc.tile_pool(name="xlast", bufs=1) as xlast,
            tc.tile_pool(name="expo", bufs=3) as expo,
            tc.tile_pool(name="psum", bufs=4, space="PSUM") as psum_pool,
            tc.tile_pool(name="psum_last", bufs=1, space="PSUM") as psum_last,
        ):
            # startup dead zone (engine barrier + instruction fetch lasts
            # ~7us before the first score byte lands): memsets and a warmup
            # exp that pulls the ACT_TABLE_LOAD off the first real exp's
            # critical path
            bias_t = singles.tile([P, 1], mybir.dt.float32)
            nc.vector.memset(bias_t[:], EXP_SHIFT)
            warm = singles.tile([P, 1], mybir.dt.float32)
            nc.scalar.activation(out=warm[:], in_=bias_t[:],
                                 func=mybir.ActivationFunctionType.Exp,
                                 bias=bias_t[:], scale=1.0)
            # all per-group stats accumulate here; one DMA ships them at the
            # end (one sem slot instead of eight)
            oball = singles.tile([16, NGROUP + 1, 3], mybir.dt.float32)
            nc.vector.memset(oball[:], 0.0)
            tmp = singles.tile([16, 512], mybir.dt.float32)
            junk = singles.tile([16, 256], mybir.dt.float32)

            # weights go through the idle gpsimd SWDGE so the sync ring's
            # very first descriptor is score data (PE doesn't need them
            # until ~16us in)
            wt = singles.tile([P, NCHUNK + 16, 16], MM_DT)
            nc.gpsimd.dma_start(out=wt[:], in_=wmat[:])
            wvec = singles.tile([16, 2, 256], mybir.dt.float32)
            nc.gpsimd.dma_start(out=wvec[:], in_=wvin[:])

            xts = []
            for g in range(2):
                xt = xin.tile([P, FD], mybir.dt.float32)
                nc.sync.dma_start(out=xt[:], in_=score[g * P:(g + 1) * P, :])
                xts.append(xt)

            wv256 = wvec.rearrange("p a b -> p (a b)")  # [16, 512] = iota%256

            def finalize_zx(ps_sl, slot, cw, use_act):
                """Z/S16 + Xw of one PSUM window into oball[:, slot, :].

                use_act: Z via ACT Copy+accum (parallel with the DVE chain;
                right for the drain, where ACT is otherwise done). Mid-stream
                ACT must keep chasing exps, so window A sums Z on DVE."""
                # DVE op first in program order: the scheduler sequences
                # same-PSUM readers by emission order, and the DVE leg is
                # the longer one — emitting it first lets ACT's Copy+accum
                # run concurrently instead of gating it
                nc.vector.tensor_mul(tmp[:, 0:cw], ps_sl, wv256[:, 0:cw])
                nc.vector.reduce_sum(oball[:, slot, 2:3], tmp[:, 0:cw],
                                     axis=mybir.AxisListType.X)
                if use_act:
                    nc.scalar.activation(out=junk[:, 0:cw], in_=ps_sl,
                                         func=mybir.ActivationFunctionType.Copy,
                                         accum_out=oball[:, slot, 0:1])
                else:
                    nc.vector.reduce_sum(oball[:, slot, 0:1], ps_sl,
                                         axis=mybir.AxisListType.X)

            for g in range(NGROUP - 1):
                if g < 2:
                    xt = xts[g]
                else:
                    xt = xin.tile([P, FD], mybir.dt.float32)
                    nc.sync.dma_start(out=xt[:], in_=score[g * P:(g + 1) * P, :])
                et = expo.tile([P, FD], MM_DT)
                ps = psum_pool.tile([16, 512], mybir.dt.float32)
                cw = FD // NCHUNK
                for a in range(2):
                    astep = FD // 2
                    nc.scalar.activation(out=et[:, a * astep:(a + 1) * astep],
                                         in_=xt[:, a * astep:(a + 1) * astep],
                                         func=mybir.ActivationFunctionType.Exp,
                                         bias=bias_t[:], scale=1.0 / TEMPERATURE)
                    for q in range(a * 4, a * 4 + 4):
                        nc.tensor.matmul(
                            ps[:, 0:cw], wt[:, q, :], et[:, q * cw:(q + 1) * cw],
                            start=(q == 0), stop=(q == NCHUNK - 1),
                        )
                nc.vector.reduce_sum(oball[:, g, 0:1], ps[:, 0:cw],
                                     axis=mybir.AxisListType.X)
                nc.vector.reduce_sum(oball[:, g, 1:2], ps[:, 256:512],
                                     axis=mybir.AxisListType.X)
                nc.vector.tensor_mul(tmp[:, 0:cw], ps[:, 0:cw], wv256[:, 0:cw])
                nc.vector.reduce_sum(oball[:, g, 2:3], tmp[:, 0:cw],
                                     axis=mybir.AxisListType.X)

            # Last group: lands in 512-col pieces (own pool: fresh sem slots,
            # so no anti-alias wait on g5/g6 completion delays the descriptor
            # issue and starves the engines), final piece split 256+256 so the
            # post-stream drain holds only one 256-col exp. 256-wide matmul
            # chunks whose weights absorb h_low entirely (no B term), split
            # into two PSUM accumulation windows: window A (chunks 0-7)
            # finalizes mid-stream into slot 7, window B (chunks 8-15)
            # finalizes in the drain into slot 8; the host adds the slots.
            g = NGROUP - 1
            xt = xlast.tile([P, FD], mybir.dt.float32)
            bounds = [0, 512, 1024, 1536, 2048, 2560, 3072, 3584, 3840, 4096]
            for dd in range(len(bounds) - 1):
                nc.sync.dma_start(out=xt[:, bounds[dd]:bounds[dd + 1]],
                                  in_=score[g * P:(g + 1) * P,
                                            bounds[dd]:bounds[dd + 1]])
            et = expo.tile([P, FD], MM_DT)
            psA = psum_last.tile([16, 256], mybir.dt.float32)
            psB = psum_last.tile([16, 256], mybir.dt.float32)
            for a in range(len(bounds) - 1):
                lo, hi = bounds[a], bounds[a + 1]
                nc.scalar.activation(out=et[:, lo:hi], in_=xt[:, lo:hi],
                                     func=mybir.ActivationFunctionType.Exp,
                                     bias=bias_t[:], scale=1.0 / TEMPERATURE)
                for q in range(lo // 256, hi // 256):
                    nc.tensor.matmul(
                        (psA if q < 8 else psB)[:, 0:256], wt[:, NCHUNK + q, :],
                        et[:, q * 256:(q + 1) * 256],
                        start=(q % 8 == 0), stop=(q % 8 == 7),
                    )
                if hi == 2048:
                    finalize_zx(psA[:, 0:256], NGROUP - 1, 256, use_act=False)
            finalize_zx(psB[:, 0:256], NGROUP, 256, use_act=True)

            nc.sync.dma_start(out=stats[:], in_=oball[:])

    nc.compile()
    return nc


def _weights():
    W = np.zeros((P, NCHUNK + 16, 16), dtype=np.float32)
    s = np.arange(16)
    for j in range(NPG):
        W[16 * j + s, :, 2 * j] = 1.0
        for q in range(NCHUNK):          # 512-wide chunks: h_low = 2q + b
            W[16 * j + s, q, 2 * j + 1] = (16 * s + 2 * q).astype(np.float32)
        for q in range(16):              # 256-wide chunks: h_low = q
            W[16 * j + s, NCHUNK + q, 2 * j + 1] = (16 * s + q).astype(np.float32)
    return W


def _get_compiled():
    if "nc" not in _CACHE:
        _CACHE["nc"] = _build_nc()
        _CACHE["W"] = _weights()
        wv = np.tile(np.arange(256, dtype=np.float32)[None, None, :], (16, 2, 1))
        _CACHE["WV"] = wv
    return _CACHE["nc"], _CACHE["W"]


def _device_coords(score_map, trace=False):
    """Run the Bass kernel on 8 cores; return y, x arrays of shape [BS, K]
    (plus the BassKernelResults of the run)."""
    nc, W = _get_compiled()
    flat = np.ascontiguousarray(score_map.reshape(BS * K, 16, FD))
    bpc = BS // N_CORES                      # batches per core
    mpc = bpc * K                            # maps per core
    in_maps = []
    for c in range(N_CORES):
        shard = flat[c * mpc:(c + 1) * mpc].reshape(NGROUP * P, FD)
        in_maps.append({"score": shard, "wmat": W, "wvin": _CACHE["WV"]})
    res = run_bass_kernel_spmd(nc, in_maps, list(range(N_CORES)), trace=trace)

    ys = np.empty((BS, K), dtype=np.float64)
    xs = np.empty((BS, K), dtype=np.float64)
    for c in range(N_CORES):
        st = res.results[c]["stats"].astype(np.float64)   # [16, 9, 3]
        rows = np.arange(NPG)
        # slot 8 is the last group's second PSUM window; fold it into slot 7
        st = st[:, :NGROUP, :] + np.pad(
            st[:, NGROUP:, :], ((0, 0), (NGROUP - 1, 0), (0, 0)))
        Z = st[2 * rows, :, 0]               # [j, g]
        S16 = st[2 * rows + 1, :, 0]
        B = st[2 * rows, :, 1].copy()
        B[:, NGROUP - 1] = 0.0           # last group: h_low fully in S16
        Xw = st[2 * rows, :, 2]
        y = (S16 + B) / Z                    # [j, g]
        x = Xw / Z
        # map (g, j) -> core-local pair index 8g + j -> (b_local, k)
        y = y.T.reshape(mpc)                 # [g, j] -> pair-major
        x = x.T.reshape(mpc)
        ys[c * bpc:(c + 1) * bpc] = y.reshape(bpc, K)
        xs[c * bpc:(c + 1) * bpc] = x.reshape(bpc, K)
    return ys, xs, res


def _exact_coords(sm64):
    """Float64 softmax expected coords for one [HO, WO] score map."""
    z = sm64 / TEMPERATURE
    z = z - z.max()
    e = np.exp(z)
    Z = e.sum()
    y = (e.sum(axis=1) * np.arange(HO)).sum() / Z
    x = (e.sum(axis=0) * np.arange(WO)).sum() / Z
    return y, x


def _soft_ceil(x):
    return x + (1.0 - 1.0 / (1.0 + np.exp(-SHARPNESS * (x - np.floor(x)))))


def _smooth_clamp(x, min_val, max_val):
    x = np.where(x < min_val,
                 min_val + SMOOTHNESS * np.tanh((x - min_val) / SMOOTHNESS), x)
    x = np.where(x > max_val,
                 max_val - SMOOTHNESS * np.tanh((max_val - x) / SMOOTHNESS), x)
    return x


def kernel(score_map, offset_map, _trace=False, _res_out=None):
    score_map = np.asarray(score_map)
    offset_map = np.asarray(offset_map)

    ys, xs, res = _device_coords(score_map, trace=_trace)
    if _res_out is not None:
        _res_out.append(res)

    coords = np.stack([ys, xs], axis=-1)          # [BS, K, 2] float64

    cc = _soft_ceil(coords)
    y_cl = _smooth_clamp(cc[..., 0], 0.0, float(HO - 1))
    x_cl = _smooth_clamp(cc[..., 1], 0.0, float(WO - 1))

    # The harness executes the reference on the same neuron jax backend,
    # where .astype(int32) rounds half-to-even (np.rint) rather than
    # truncating. Refine pairs whose clamped coords sit near a rounding
    # boundary (half-integers): the cast there is sensitive to the
    # device's float32r noise.
    fy = np.abs(y_cl - np.floor(y_cl) - 0.5)
    fx = np.abs(x_cl - np.floor(x_cl) - 0.5)
    sus = (fy < REFINE_DELTA) | (fx < REFINE_DELTA)
    for b, k in zip(*np.nonzero(sus)):
        yy, xx = _exact_coords(score_map[b, k].astype(np.float64))
        coords[b, k, 0] = yy
        coords[b, k, 1] = xx
        cc = _soft_ceil(coords[b, k])
        y_cl[b, k] = _smooth_clamp(cc[0], 0.0, float(HO - 1))
        x_cl[b, k] = _smooth_clamp(cc[1], 0.0, float(WO - 1))

    y_idx = np.rint(y_cl).astype(np.int32)
    x_idx = np.rint(x_cl).astype(np.int32)
    b_idx = np.arange(BS)[:, None]
    k_idx = np.arange(K)[None, :]
    off_y = offset_map[b_idx, 2 * k_idx, y_idx, x_idx]
    off_x = offset_map[b_idx, 2 * k_idx + 1, y_idx, x_idx]
    offset = np.stack([off_y, off_x], axis=-1)

    pts = (coords.astype(np.float32) + offset) * STRIDE
    return pts.astype(np.float32)

